# revision 22
# baseline (speedup 1.0000x reference)
"""Trainium2 Bass kernel for AllPassMORRCirculantLinear.

Math (reference, per batch row b):
  xb = x.reshape(bs, q, k); xb = xb*xb
  phi[b,p,q,t] = sum_s xb[b,q,s] * |w|[p,q,(t-s) mod k]   (circular conv, k=8)
  t(phi) = (a^2 + r^2 - 2 a r cos phi) / (1 + (ar)^2 - 2 a r cos phi)
  out[b, p*k+t] = sum_q scale[q] * t(phi[b,p,q,t])

With rho = a*r, A = (1-rho)^2, K = (1-a^2)(1-r^2), s'_q = -K*scale_q and
sum_q scale_q == 0:
  out = sum_q s'_q * u_q,  u_q = 1/(A + 4 rho sin^2(phi_q/2))

Pipeline (data-parallel over batch, 128 rows/core), per q:
  PE    : h = phi/(2pi) in fp32 PSUM via ONE 24-row stacked fp16 matmul
          per 512-chunk (rows = [xh, xl, xh] vs [wh, wh, wl]; weights
          pre-scaled by 1/2pi).  sin^2(pi*h) has period 1 in h, so range
          reduction is an EXACT integer subtract -- no Cody-Waite.
  DVE   : one fused 8-stage custom op SINSQ_FRAC_ANT:
            g = h - round(h)  (magic-number round, 3 stages)
            v = ((C0*g^2 + C1)*g)^2  (weighted-minimax squared-cubic
                ~ sin^2(pi g); its free additive constant rides the ACT
                reciprocal bias).  fp16 out.
  ACT   : u'_q = Reciprocal(SCL_q * v + BIA_q) = s'_q * u_q with
          SCL_q = 4 rho/s'_q, BIA_q = (A + 4 rho c)/s'_q baked as
          immediates (compile is keyed on the scale values); fp16 out.
          Square (x^2 staging) + Reciprocal live in ONE table set
          (reciprocal_and_small): zero ACT table switches.
  PE    : accumulation as identity-stationary matmuls into a PSUM
          region held across the iteration: acc[b,:] += I @ u'_q.
          fp16 moving operand streams 1 col/cycle; PSUM accumulates in
          fp32 (exact).  Readout = one ACT copy PSUM->SBUF per iter.
GPSIMD idle; DVE carries only the custom op; engines balance ~150us.
"""

import sys

for _p in ("/opt/trn_rl_repo",):
    if _p not in sys.path:
        sys.path.insert(0, _p)

import hashlib
import numpy as np
from contextlib import ExitStack

MRR_A = 0.8682
MRR_R = 0.8602
RHO = MRR_A * MRR_R
ACONST = (1.0 - RHO) ** 2
KCONST = (1.0 - MRR_A * MRR_A) * (1.0 - MRR_R * MRR_R)

BS, IN_CH, OUT_CH, KB = 1024, 1024, 1024, 8
Q = IN_CH // KB    # 128
P = OUT_CH // KB   # 128
NCORES = 8
BSC = BS // NCORES  # 128 batch rows per core

MAGIC = 12582912.0  # 1.5 * 2**23: y + MAGIC - MAGIC == round(y) in fp32 RNE
# weighted-minimax squared-cubic: sin^2(pi g) ~ (C1*g + C0*g^3)^2 + CB,
# weight 4 rho/d^2 (error measured in du/u); peak |du/u| = 7.9e-3
SINSQ_C0 = -4.64495414
SINSQ_C1 = 3.13652214
SINSQ_CB = 1.08338380e-05

# Accumulation split.  All-PE acc (identity matmuls) makes PE's rate ~=
# DVE's rate and the PE->DVE->ACT->PE loop then loses ~80us to coupling
# stalls (PE wait-queue is 4 deep; DVE has only 3 PSUM h-slots of
# buffer).  Giving the odd quads to the otherwise-idle GPSIMD keeps PE
# well under DVE's rate so the pipeline never starves: even quads of 4
# q's -> PE PSUM matmul acc (batched one group behind), odd quads ->
# ACT writes a [128, 4096] fp16 quad which GPSIMD adds into a 4-phase
# fp16 accumulator.
ACC_GROUP = 16
# which quads (mod 4) go to GPSIMD; the rest go to PE.  Alternating
# spreads the GP adds evenly -- clustered assignments measure worse.
import os
GP_MASK = {1: (3,), 2: (1, 3), 3: (1, 2, 3)}[
    int(os.environ.get("GP_QUADS_OF_4", "2"))]

_CACHE = {}


def _sinsq_ref(in0, in1, s0, s1, imm2):
    f = np.float32
    t = (in0.astype(f) + f(imm2)).astype(f)
    k = (t - f(imm2)).astype(f)
    g = (in0.astype(f) - k).astype(f)
    y = (g * g).astype(f)
    p = ((f(s0) * y).astype(f) + f(s1)).astype(f)
    e = (p * g).astype(f)
    return (e * e).astype(f)


def _register_sinsq():
    """Custom DVE op: v = ((C0*g^2 + C1)*g)^2, g = x - round(x). 8 stages."""
    from concourse import dve_ops
    from concourse.dve_spec import Spec, Src0, C0, C1, C2, sq, lower
    from concourse.dve_uop import DveOpSpec

    name = "SINSQ_FRAC_ANT"
    if name in dve_ops._SUB_OPCODE_FOR_NAME:
        return next(op for op in dve_ops.OPS if op.name == name)
    g = Src0 - ((Src0 + C2) - C2)
    e = (C0 * sq(g) + C1) * g
    spec = Spec(body=sq(e), reference=_sinsq_ref)
    row = max(dve_ops._SUB_OPCODE_FOR_NAME.values()) + 1
    assert row < 0x20
    dve_ops._SUB_OPCODE_FOR_NAME[name] = row
    shas = {}
    for ver in ("v3", "v4"):
        c = DveOpSpec(name=name, opcode=row, uops=lower(spec, ver=ver))
        shas[ver] = c.sha(ver)
    op = dve_ops.DveOp(name, spec, subdim=False, uops_sha=shas)
    dve_ops.OPS.append(op)
    dve_ops.CUSTOM_DVE_SPECS[name] = spec
    return op


def _raw_recip(nc, out, in_, bias, scale):
    """ACT Reciprocal (bypasses the bass low-precision guard; measured
    ~1.2e-5 max rel on TRN2 HW over the +-[0.3, 2000] range used here)."""
    from concourse import mybir

    eng = nc.scalar
    inputs = [eng.lower_ap(in_)]
    for arg in (bias, scale, 0.0):
        inputs.append(mybir.ImmediateValue(dtype=mybir.dt.float32, value=float(arg)))
    return eng.add_instruction(
        mybir.InstActivation(
            name=nc.get_next_instruction_name(),
            func=mybir.ActivationFunctionType.Reciprocal,
            ins=inputs,
            outs=[eng.lower_ap(out)],
        )
    )


def _build_nc(sprime, niter=1):
    from concourse import bacc, mybir
    import concourse.tile as tile
    from concourse import masks

    sinsq_op = _register_sinsq()

    nc = bacc.Bacc("TRN2", debug=False)
    f32 = mybir.dt.float32
    f16 = mybir.dt.float16
    AF = mybir.ActivationFunctionType

    x_d = nc.dram_tensor("x", [BSC, IN_CH], f32, kind="ExternalInput")
    # stacked circulant moving operand, pre-scaled by 1/(2 pi):
    # rows 0..7 wh (vs xh), 8..15 wh (vs xl), 16..23 wl (vs xh)
    wst_d = nc.dram_tensor("wst", [3 * KB, Q, OUT_CH], f16, kind="ExternalInput")
    out_d = nc.dram_tensor("out", [BSC, OUT_CH], f32, kind="ExternalOutput")

    scl = (4.0 * RHO / sprime).astype(np.float64)
    bia = ((ACONST + 4.0 * RHO * SINSQ_CB) / sprime).astype(np.float64)

    with tile.TileContext(nc) as tc:
        with ExitStack() as ctx:
            singles = ctx.enter_context(tc.tile_pool(name="singles", bufs=1))
            # h singles [128, 1024] = 2 banks * bufs=3 + acc 2 banks = 8
            psum = ctx.enter_context(tc.tile_pool(name="psum", bufs=3, space="PSUM"))
            apsum = ctx.enter_context(tc.tile_pool(name="apsum", bufs=1, space="PSUM"))
            wqp = ctx.enter_context(tc.tile_pool(name="wqp", bufs=2))
            vpool = ctx.enter_context(tc.tile_pool(name="vpool", bufs=8))
            upool = ctx.enter_context(tc.tile_pool(name="upool",
                                                   bufs=ACC_GROUP + 10))
            gqpool = ctx.enter_context(tc.tile_pool(name="gqpool", bufs=4))

            identf = singles.tile([128, 128], f32)
            masks.make_identity(nc, identf[:])
            ident = singles.tile([128, 128], f16)
            nc.vector.tensor_copy(ident[:], identf[:])

            x_sb = singles.tile([128, IN_CH], f32)
            nc.sync.dma_start(x_sb[:], x_d.ap())
            # input intensity modulation: x <- x^2 (in place).  Square is
            # in reciprocal_and_small: same table set as Reciprocal.
            nc.scalar.activation(x_sb[:], x_sb[:], AF.Square)

            # staged squared-transposed x in fp16 hi/lo (stationary):
            # rows 0..7 = xh, rows 8..15 = xl, rows 16..23 = xh (dup)
            xsts = []
            xlp = ctx.enter_context(tc.tile_pool(name="xlp", bufs=2))
            for g in range(8):
                xst = singles.tile([3 * KB, 16, 128], f16, tag=f"xst{g}")
                for half in range(2):
                    xtp = psum.tile([128, OUT_CH], f32, tag="h")
                    for j8 in range(8):
                        j = half * 8 + j8
                        nc.tensor.transpose(
                            xtp[0:8, j8 * 128:(j8 + 1) * 128],
                            x_sb[:, (g * 16 + j) * 8:(g * 16 + j) * 8 + 8],
                            identf[:])
                    xsl = xst[0:8, half * 8:(half + 1) * 8, :]
                    nc.scalar.copy(xsl, xtp[0:8, :].rearrange("s (j b) -> s j b", j=8))
                    xl_tmp = xlp.tile([8, 8 * 128], f16)
                    nc.vector.tensor_sub(xl_tmp[:], xtp[0:8, :],
                                         xsl.rearrange("s j b -> s (j b)"))
                    nc.scalar.dma_start(
                        xst[8:16, half * 8:(half + 1) * 8, :]
                        .rearrange("s j b -> s (j b)"), xl_tmp[:])
                    nc.scalar.dma_start(
                        xst[16:24, half * 8:(half + 1) * 8, :]
                        .rearrange("s j b -> s (j b)"),
                        xsl.rearrange("s j b -> s (j b)"))
                xsts.append(xst)

            out_sb = singles.tile([128, OUT_CH], f32)
            # GPSIMD-side 4-phase fp16 accumulator (odd quads)
            acc_g = singles.tile([128, 4 * OUT_CH], f16)

            wq_tiles = {}

            def wq_fetch(c):  # c = 8-q chunk index, 16 per iteration
                wq = wqp.tile([3 * KB, 8, OUT_CH], f16, tag="wq")
                nc.sync.dma_start(wq[:], wst_d.ap()[:, c * 8:(c + 1) * 8, :])
                wq_tiles[c % 2] = wq

            def front_end(q, up_dst):
                """PE h-matmul -> DVE sinsq -> ACT recip into up_dst."""
                g, rem = divmod(q, 16)
                c, crem = divmod(q, 8)
                if crem == 0 and c + 1 < 16:
                    wq_fetch(c + 1)  # prefetch next chunk (bufs=2)
                wq = wq_tiles[c % 2]
                hps = psum.tile([128, OUT_CH], f32, tag="h")
                for hc in range(2):
                    nc.tensor.matmul(
                        hps[:, hc * 512:(hc + 1) * 512],
                        xsts[g][:, rem, :],
                        wq[:, crem, hc * 512:(hc + 1) * 512],
                        start=True, stop=True,
                    )
                vt = vpool.tile([128, OUT_CH], f16, tag="v")
                nc.vector._custom_dve(
                    sinsq_op, out=vt[:], in0=hps[:],
                    s0=SINSQ_C0, s1=SINSQ_C1, imm2=MAGIC)
                _raw_recip(nc, up_dst, vt[:], bias=bia[q], scale=scl[q])

            def acc_mm(acc, up, first, last):
                """PE: acc += I @ u'_q (PSUM accumulate, fp16 moving)."""
                for hc in range(2):
                    nc.tensor.matmul(
                        acc[:, hc * 512:(hc + 1) * 512],
                        ident[:],
                        up[:, hc * 512:(hc + 1) * 512],
                        start=first, stop=last,
                        skip_group_check=True,
                    )

            def run_iter():
                acc = apsum.tile([128, OUT_CH], f32, tag="acc")
                wq_fetch(0)
                ups = {}       # PE-destined u' tiles by q
                n_pe_total = 32 * (4 - len(GP_MASK))
                n_pe_acc = 0
                gp_first = True
                for g in range(Q // ACC_GROUP):
                    gbase = g * ACC_GROUP
                    for quad in range(ACC_GROUP // 4):
                        q0 = gbase + quad * 4
                        if quad % 4 not in GP_MASK:
                            # PE quad: individual u' tiles, acc'd next group
                            for q in range(q0, q0 + 4):
                                up = upool.tile([128, OUT_CH], f16, tag="u")
                                front_end(q, up[:])
                                ups[q] = up
                        else:
                            # GPSIMD quad: 4 q's into one wide fp16 tile
                            gq = gqpool.tile([128, 4 * OUT_CH], f16, tag="gq")
                            for j, q in enumerate(range(q0, q0 + 4)):
                                front_end(q, gq[:, j * OUT_CH:(j + 1) * OUT_CH])
                            if gp_first:
                                nc.gpsimd.tensor_copy(acc_g[:], gq[:])
                                gp_first = False
                            else:
                                nc.gpsimd.tensor_add(acc_g[:], acc_g[:], gq[:])
                    if g > 0:
                        for q in sorted(ups):
                            if q >= gbase:
                                break
                            acc_mm(acc, ups.pop(q)[:], n_pe_acc == 0,
                                   n_pe_acc == n_pe_total - 1)
                            n_pe_acc += 1
                for q in sorted(ups):
                    acc_mm(acc, ups.pop(q)[:], n_pe_acc == 0,
                           n_pe_acc == n_pe_total - 1)
                    n_pe_acc += 1
                nc.scalar.copy(out_sb[:], acc[:])

            if niter == 1:
                run_iter()
            else:
                with tc.For_i(0, niter, 1):
                    run_iter()

            # merge the GPSIMD-side phases (outside the loop)
            ph0 = singles.tile([128, OUT_CH], f32)
            ph1 = singles.tile([128, OUT_CH], f32)
            nc.vector.tensor_add(ph0[:], acc_g[:, 0:OUT_CH],
                                 acc_g[:, OUT_CH:2 * OUT_CH])
            nc.vector.tensor_add(ph1[:], acc_g[:, 2 * OUT_CH:3 * OUT_CH],
                                 acc_g[:, 3 * OUT_CH:4 * OUT_CH])
            nc.vector.tensor_add(ph0[:], ph0[:], ph1[:])
            nc.vector.tensor_add(out_sb[:], out_sb[:], ph0[:])
            nc.sync.dma_start(out_d.ap(), out_sb[:])

    nc.compile()
    return nc


def _host_prep(weight, morr_output_scale):
    w = np.abs(np.asarray(weight, dtype=np.float32))   # [P, Q, KB]
    s = np.asarray(morr_output_scale, dtype=np.float32)
    s = s - s.mean()
    half = s[..., :-1, :]                              # [1,1,Q//2,1]
    scale = np.concatenate([half, -half], axis=2)[0, 0, :, 0].astype(np.float32)
    sprime = (-KCONST * scale).astype(np.float32)      # folded -K

    # circulant moving-operand layout: wc[s, q, p*KB+t] = w[p, q, (t-s) % KB],
    # pre-scaled by 1/(2 pi) so the matmul yields h = phi/(2 pi)
    wc = np.empty((KB, Q, P * KB), np.float32)
    for sh in range(KB):
        rolled = np.roll(w, sh, axis=2)
        wc[sh] = rolled.transpose(1, 0, 2).reshape(Q, P * KB)
    wc /= np.float32(2.0 * np.pi)

    wh = wc.astype(np.float16)
    wl = (wc - wh.astype(np.float32)).astype(np.float16)
    wst = np.empty((3 * KB, Q, P * KB), np.float16)
    wst[0:KB] = wh
    wst[KB:2 * KB] = wh
    wst[2 * KB:3 * KB] = wl
    return wst, sprime


def kernel(x, weight, morr_output_scale, _trace=False):
    from concourse import bass_utils

    wst, sprime = _host_prep(weight, morr_output_scale)
    key = hashlib.sha1(sprime.tobytes()).hexdigest()
    if _CACHE.get("key") != key:
        _CACHE["nc"] = _build_nc(sprime)
        _CACHE["key"] = key
    nc = _CACHE["nc"]

    x = np.ascontiguousarray(np.asarray(x, dtype=np.float32))
    in_maps = []
    for c in range(NCORES):
        in_maps.append({
            "x": np.ascontiguousarray(x[c * BSC:(c + 1) * BSC]),
            "wst": wst,
        })
    res = bass_utils.run_bass_kernel_spmd(
        nc, in_maps, core_ids=list(range(NCORES)), trace=_trace)
    out = np.concatenate([res.results[c]["out"] for c in range(NCORES)], axis=0)
    if _trace:
        _CACHE["last_results"] = res
    return out


# revision 29
# speedup vs baseline: 1.2113x; 1.2113x over previous
"""Trainium2 Bass kernel for AllPassMORRCirculantLinear.

Math (reference, per batch row b):
  xb = x.reshape(bs, q, k); xb = xb*xb
  phi[b,p,q,t] = sum_s xb[b,q,s] * |w|[p,q,(t-s) mod k]   (circular conv, k=8)
  t(phi) = (a^2 + r^2 - 2 a r cos phi) / (1 + (ar)^2 - 2 a r cos phi)
  out[b, p*k+t] = sum_q scale[q] * t(phi[b,p,q,t])

With rho = a*r, A = (1-rho)^2, K = (1-a^2)(1-r^2), s'_q = -K*scale_q and
sum_q scale_q == 0:
  out = sum_q s'_q * u_q,  u_q = 1/(A + 4 rho sin^2(phi_q/2))

Pipeline (data-parallel over batch, 128 rows/core), per q:
  PE    : h = phi/(2pi) in fp32 PSUM via ONE 24-row stacked fp16 matmul
          per 512-chunk (rows = [xh, xl, xh] vs [wh, wh, wl]; weights
          pre-scaled by 1/2pi).  sin^2(pi*h) has period 1 in h, so range
          reduction is an EXACT integer subtract -- no Cody-Waite.
  DVE   : one fused 8-stage custom op SINSQ_FRAC_ANT:
            g = h - round(h)  (magic-number round, 3 stages)
            v = ((C0*g^2 + C1)*g)^2  (weighted-minimax squared-cubic
                ~ sin^2(pi g); its free additive constant rides the ACT
                reciprocal bias).  fp16 out.
  ACT   : u'_q = Reciprocal(SCL_q * v + BIA_q) = s'_q * u_q with
          SCL_q = 4 rho/s'_q, BIA_q = (A + 4 rho c)/s'_q baked as
          immediates (compile is keyed on the scale values); fp16 out.
          Square (x^2 staging) + Reciprocal live in ONE table set
          (reciprocal_and_small): zero ACT table switches.
  PE    : accumulation as identity-stationary matmuls into a PSUM
          region held across the iteration: acc[b,:] += I @ u'_q.
          fp16 moving operand streams 1 col/cycle; PSUM accumulates in
          fp32 (exact).  Readout = one ACT copy PSUM->SBUF per iter.
GPSIMD idle; DVE carries only the custom op; engines balance ~150us.
"""

import sys

for _p in ("/opt/trn_rl_repo",):
    if _p not in sys.path:
        sys.path.insert(0, _p)

import hashlib
import numpy as np
from contextlib import ExitStack

MRR_A = 0.8682
MRR_R = 0.8602
RHO = MRR_A * MRR_R
ACONST = (1.0 - RHO) ** 2
KCONST = (1.0 - MRR_A * MRR_A) * (1.0 - MRR_R * MRR_R)

BS, IN_CH, OUT_CH, KB = 1024, 1024, 1024, 8
Q = IN_CH // KB    # 128
P = OUT_CH // KB   # 128
NCORES = 8
BSC = BS // NCORES  # 128 batch rows per core

MAGIC = 12582912.0  # 1.5 * 2**23: y + MAGIC - MAGIC == round(y) in fp32 RNE
# weighted-minimax squared-cubic: sin^2(pi g) ~ (C1*g + C0*g^3)^2 + CB,
# weight 4 rho/d^2 (error measured in du/u); peak |du/u| = 7.9e-3
SINSQ_C0 = -4.64495414
SINSQ_C1 = 3.13652214
SINSQ_CB = 1.08338380e-05

# Accumulation split.  All-PE acc (identity matmuls) makes PE's rate ~=
# DVE's rate and the PE->DVE->ACT->PE loop then loses ~80us to coupling
# stalls (PE wait-queue is 4 deep; DVE has only 3 PSUM h-slots of
# buffer).  Giving the odd quads to the otherwise-idle GPSIMD keeps PE
# well under DVE's rate so the pipeline never starves: even quads of 4
# q's -> PE PSUM matmul acc (batched one group behind), odd quads ->
# ACT writes a [128, 4096] fp16 quad which GPSIMD adds into a 4-phase
# fp16 accumulator.
ACC_GROUP = 16
# which quads (mod 4) go to GPSIMD; the rest go to PE.  Alternating
# spreads the GP adds evenly -- clustered assignments measure worse.
import os
GP_MASK = {1: (3,), 2: (1, 3), 3: (1, 2, 3)}[
    int(os.environ.get("GP_QUADS_OF_4", "2"))]

_CACHE = {}


def _sinsq_ref(in0, in1, s0, s1, imm2):
    f = np.float32
    t = (in0.astype(f) + f(imm2)).astype(f)
    k = (t - f(imm2)).astype(f)
    g = (in0.astype(f) - k).astype(f)
    y = (g * g).astype(f)
    p = ((f(s0) * y).astype(f) + f(s1)).astype(f)
    e = (p * g).astype(f)
    return (e * e).astype(f)


def _register_sinsq():
    """Custom DVE op: v = ((C0*g^2 + C1)*g)^2, g = x - round(x). 8 stages."""
    from concourse import dve_ops
    from concourse.dve_spec import Spec, Src0, C0, C1, C2, sq, lower
    from concourse.dve_uop import DveOpSpec

    name = "SINSQ_FRAC_ANT"
    if name in dve_ops._SUB_OPCODE_FOR_NAME:
        return next(op for op in dve_ops.OPS if op.name == name)
    g = Src0 - ((Src0 + C2) - C2)
    e = (C0 * sq(g) + C1) * g
    spec = Spec(body=sq(e), reference=_sinsq_ref)
    row = max(dve_ops._SUB_OPCODE_FOR_NAME.values()) + 1
    assert row < 0x20
    dve_ops._SUB_OPCODE_FOR_NAME[name] = row
    shas = {}
    for ver in ("v3", "v4"):
        c = DveOpSpec(name=name, opcode=row, uops=lower(spec, ver=ver))
        shas[ver] = c.sha(ver)
    op = dve_ops.DveOp(name, spec, subdim=False, uops_sha=shas)
    dve_ops.OPS.append(op)
    dve_ops.CUSTOM_DVE_SPECS[name] = spec
    return op


def _raw_recip(nc, out, in_, bias, scale):
    """ACT Reciprocal (bypasses the bass low-precision guard; measured
    ~1.2e-5 max rel on TRN2 HW over the +-[0.3, 2000] range used here)."""
    from concourse import mybir

    eng = nc.scalar
    inputs = [eng.lower_ap(in_)]
    for arg in (bias, scale, 0.0):
        inputs.append(mybir.ImmediateValue(dtype=mybir.dt.float32, value=float(arg)))
    return eng.add_instruction(
        mybir.InstActivation(
            name=nc.get_next_instruction_name(),
            func=mybir.ActivationFunctionType.Reciprocal,
            ins=inputs,
            outs=[eng.lower_ap(out)],
        )
    )


def _build_nc(sprime, niter=1):
    from concourse import bacc, mybir
    import concourse.tile as tile
    from concourse import masks

    sinsq_op = _register_sinsq()

    nc = bacc.Bacc("TRN2", debug=False)
    f32 = mybir.dt.float32
    f16 = mybir.dt.float16
    AF = mybir.ActivationFunctionType

    x_d = nc.dram_tensor("x", [BSC, IN_CH], f32, kind="ExternalInput")
    # stacked circulant moving operand, pre-scaled by 1/(2 pi):
    # rows 0..7 wh (vs xh), 8..15 wh (vs xl), 16..23 wl (vs xh)
    wst_d = nc.dram_tensor("wst", [3 * KB, Q, OUT_CH], f16, kind="ExternalInput")
    out_d = nc.dram_tensor("out", [BSC, OUT_CH], f32, kind="ExternalOutput")

    # per-rail immediates (rails: s'_{q+64} = -s'_q exactly)
    assert np.array_equal(sprime[64:], -sprime[:64])
    scl = (4.0 * RHO / sprime[:64]).astype(np.float64)
    bia = ((ACONST + 4.0 * RHO * SINSQ_CB) / sprime[:64]).astype(np.float64)

    with tile.TileContext(nc) as tc:
        with ExitStack() as ctx:
            singles = ctx.enter_context(tc.tile_pool(name="singles", bufs=1))
            # h singles [128, 1024] = 2 banks * bufs=3 + acc 2 banks = 8
            psum = ctx.enter_context(tc.tile_pool(name="psum", bufs=3, space="PSUM"))
            apsum = ctx.enter_context(tc.tile_pool(name="apsum", bufs=1, space="PSUM"))
            wqp = ctx.enter_context(tc.tile_pool(name="wqp", bufs=2))
            vpool = ctx.enter_context(tc.tile_pool(name="vpool", bufs=6))
            upool = ctx.enter_context(tc.tile_pool(name="upool", bufs=12))
            gqpool = ctx.enter_context(tc.tile_pool(name="gqpool", bufs=4))

            identf = singles.tile([128, 128], f32)
            masks.make_identity(nc, identf[:])
            ident = singles.tile([128, 128], f16)
            nc.vector.tensor_copy(ident[:], identf[:])
            identn = singles.tile([128, 128], f16)
            nc.vector.tensor_scalar_mul(identn[:], identf[:], -1.0)

            x_sb = singles.tile([128, IN_CH], f32)
            nc.sync.dma_start(x_sb[:], x_d.ap())
            # input intensity modulation: x <- x^2 (in place).  Square is
            # in reciprocal_and_small: same table set as Reciprocal.
            nc.scalar.activation(x_sb[:], x_sb[:], AF.Square)

            # staged squared-transposed x in fp16 hi/lo (stationary):
            # rows 0..7 = xh, rows 8..15 = xl, rows 16..23 = xh (dup)
            xsts = []
            xlp = ctx.enter_context(tc.tile_pool(name="xlp", bufs=2))
            for g in range(8):
                xst = singles.tile([3 * KB, 16, 128], f16, tag=f"xst{g}")
                for half in range(2):
                    xtp = psum.tile([128, OUT_CH], f32, tag="h")
                    for j8 in range(8):
                        j = half * 8 + j8
                        nc.tensor.transpose(
                            xtp[0:8, j8 * 128:(j8 + 1) * 128],
                            x_sb[:, (g * 16 + j) * 8:(g * 16 + j) * 8 + 8],
                            identf[:])
                    xsl = xst[0:8, half * 8:(half + 1) * 8, :]
                    nc.scalar.copy(xsl, xtp[0:8, :].rearrange("s (j b) -> s j b", j=8))
                    xl_tmp = xlp.tile([8, 8 * 128], f16)
                    nc.vector.tensor_sub(xl_tmp[:], xtp[0:8, :],
                                         xsl.rearrange("s j b -> s (j b)"))
                    nc.scalar.dma_start(
                        xst[8:16, half * 8:(half + 1) * 8, :]
                        .rearrange("s j b -> s (j b)"), xl_tmp[:])
                    nc.scalar.dma_start(
                        xst[16:24, half * 8:(half + 1) * 8, :]
                        .rearrange("s j b -> s (j b)"),
                        xsl.rearrange("s j b -> s (j b)"))
                xsts.append(xst)

            out_sb = singles.tile([128, OUT_CH], f32)
            # GPSIMD-side 4-phase fp16 accumulator (odd quads)
            acc_g = singles.tile([128, 4 * OUT_CH], f16)

            wq_tiles = {}

            def wq_fetch(c):  # c = 8-q chunk index, 16 per iteration
                wq = wqp.tile([3 * KB, 8, OUT_CH], f16, tag="wq")
                nc.sync.dma_start(wq[:], wst_d.ap()[:, c * 8:(c + 1) * 8, :])
                wq_tiles[c % 2] = wq

            def h_sinsq(pos, v_dst):
                """PE h-matmul -> DVE sinsq into v_dst for one position.
                Positions are rail-paired: pos 2j <-> q=j, pos 2j+1 <->
                q=j+64 (wst is host-permuted to match)."""
                q = pos // 2 if pos % 2 == 0 else 64 + pos // 2
                gx, remx = divmod(q, 16)
                c, crem = divmod(pos, 8)
                if crem == 0 and c + 1 < 16:
                    wq_fetch(c + 1)  # prefetch next chunk (bufs=2)
                wq = wq_tiles[c % 2]
                hps = psum.tile([128, OUT_CH], f32, tag="h")
                for hc in range(2):
                    nc.tensor.matmul(
                        hps[:, hc * 512:(hc + 1) * 512],
                        xsts[gx][:, remx, :],
                        wq[:, crem, hc * 512:(hc + 1) * 512],
                        start=True, stop=True,
                    )
                nc.vector._custom_dve(
                    sinsq_op, out=v_dst, in0=hps[:],
                    s0=SINSQ_C0, s1=SINSQ_C1, imm2=MAGIC)

            def pair_front(p0, up_dst):
                """Two positions (one rail pair) -> one double-wide recip:
                up_dst[:, 0:1024] = u'_q, up_dst[:, 1024:2048] = -u'_{q+64}
                (s'_{q+64} = -s'_q exactly, and 1/(-x) = -1/x)."""
                vt = vpool.tile([128, 2 * OUT_CH], f16, tag="v")
                h_sinsq(p0, vt[:, 0:OUT_CH])
                h_sinsq(p0 + 1, vt[:, OUT_CH:2 * OUT_CH])
                rail = p0 // 2
                _raw_recip(nc, up_dst, vt[:], bias=bia[rail], scale=scl[rail])

            def run_iter():
                acc = apsum.tile([128, OUT_CH], f32, tag="acc")
                wq_fetch(0)
                ups = {}       # PE-destined u' pair tiles by pair index
                n_pe_total = 32 * (4 - len(GP_MASK))  # positions on PE
                n_acc = [0]
                gp_first = True

                def acc_flush(before_pair):
                    """PE acc batch: all + halves (ident), then all -
                    halves (ident_neg) -- 2 stationary switches."""
                    pairs = [pr for pr in sorted(ups) if pr < before_pair]
                    for sgn in range(2):
                        stat = ident if sgn == 0 else identn
                        for pr in pairs:
                            up = ups[pr]
                            for hc in range(2):
                                # start/stop must hit the first/last matmul
                                # of EACH psum bank (hc 0 and 1)
                                nc.tensor.matmul(
                                    acc[:, hc * 512:(hc + 1) * 512],
                                    stat[:],
                                    up[:, sgn * OUT_CH + hc * 512:
                                       sgn * OUT_CH + (hc + 1) * 512],
                                    start=(n_acc[0] < 2),
                                    stop=(n_acc[0] >= 2 * n_pe_total - 2),
                                    skip_group_check=True,
                                )
                                n_acc[0] += 1
                    for pr in pairs:
                        ups.pop(pr)

                for g in range(Q // ACC_GROUP):
                    gbase = g * ACC_GROUP
                    for quad in range(ACC_GROUP // 4):
                        q0 = gbase + quad * 4
                        if quad % 4 not in GP_MASK:
                            # PE quad: two u' pair tiles, acc'd next group
                            for pr in (q0 // 2, q0 // 2 + 1):
                                up = upool.tile([128, 2 * OUT_CH], f16,
                                                tag="u")
                                pair_front(2 * pr, up[:])
                                ups[pr] = up
                        else:
                            # GPSIMD quad: 2 rail pairs into one wide tile;
                            # phases 0,2 hold +u', phases 1,3 hold -u'
                            gq = gqpool.tile([128, 4 * OUT_CH], f16, tag="gq")
                            for j in range(2):
                                pair_front(q0 + 2 * j,
                                           gq[:, 2 * j * OUT_CH:
                                              2 * (j + 1) * OUT_CH])
                            if gp_first:
                                nc.gpsimd.tensor_copy(acc_g[:], gq[:])
                                gp_first = False
                            else:
                                nc.gpsimd.tensor_add(acc_g[:], acc_g[:], gq[:])
                    if g > 0:
                        acc_flush(gbase // 2)
                acc_flush(Q)
                nc.scalar.copy(out_sb[:], acc[:])

            if niter == 1:
                run_iter()
            else:
                with tc.For_i(0, niter, 1):
                    run_iter()

            # merge the GPSIMD-side phases (outside the loop): phases 0,2
            # hold +u', phases 1,3 hold -u'
            ph0 = singles.tile([128, OUT_CH], f32)
            ph1 = singles.tile([128, OUT_CH], f32)
            nc.vector.tensor_add(ph0[:], acc_g[:, 0:OUT_CH],
                                 acc_g[:, 2 * OUT_CH:3 * OUT_CH])
            nc.vector.tensor_add(ph1[:], acc_g[:, OUT_CH:2 * OUT_CH],
                                 acc_g[:, 3 * OUT_CH:4 * OUT_CH])
            nc.vector.tensor_sub(ph0[:], ph0[:], ph1[:])
            nc.vector.tensor_add(out_sb[:], out_sb[:], ph0[:])
            nc.sync.dma_start(out_d.ap(), out_sb[:])

    nc.compile()
    return nc


def _host_prep(weight, morr_output_scale):
    w = np.abs(np.asarray(weight, dtype=np.float32))   # [P, Q, KB]
    s = np.asarray(morr_output_scale, dtype=np.float32)
    s = s - s.mean()
    half = s[..., :-1, :]                              # [1,1,Q//2,1]
    scale = np.concatenate([half, -half], axis=2)[0, 0, :, 0].astype(np.float32)
    sprime = (-KCONST * scale).astype(np.float32)      # folded -K

    # circulant moving-operand layout: wc[s, q, p*KB+t] = w[p, q, (t-s) % KB],
    # pre-scaled by 1/(2 pi) so the matmul yields h = phi/(2 pi)
    wc = np.empty((KB, Q, P * KB), np.float32)
    for sh in range(KB):
        rolled = np.roll(w, sh, axis=2)
        wc[sh] = rolled.transpose(1, 0, 2).reshape(Q, P * KB)
    wc /= np.float32(2.0 * np.pi)

    wh = wc.astype(np.float16)
    wl = (wc - wh.astype(np.float32)).astype(np.float16)
    wst = np.empty((3 * KB, Q, P * KB), np.float16)
    wst[0:KB] = wh
    wst[KB:2 * KB] = wh
    wst[2 * KB:3 * KB] = wl
    # rail-paired position order: [0, 64, 1, 65, ...]
    perm = np.empty(Q, np.int64)
    perm[0::2] = np.arange(Q // 2)
    perm[1::2] = np.arange(Q // 2) + Q // 2
    wst = np.ascontiguousarray(wst[:, perm, :])
    return wst, sprime


def kernel(x, weight, morr_output_scale, _trace=False):
    from concourse import bass_utils

    wst, sprime = _host_prep(weight, morr_output_scale)
    key = hashlib.sha1(sprime.tobytes()).hexdigest()
    if _CACHE.get("key") != key:
        _CACHE["nc"] = _build_nc(sprime)
        _CACHE["key"] = key
    nc = _CACHE["nc"]

    x = np.ascontiguousarray(np.asarray(x, dtype=np.float32))
    in_maps = []
    for c in range(NCORES):
        in_maps.append({
            "x": np.ascontiguousarray(x[c * BSC:(c + 1) * BSC]),
            "wst": wst,
        })
    res = bass_utils.run_bass_kernel_spmd(
        nc, in_maps, core_ids=list(range(NCORES)), trace=_trace)
    out = np.concatenate([res.results[c]["out"] for c in range(NCORES)], axis=0)
    if _trace:
        _CACHE["last_results"] = res
    return out


# revision 30
# speedup vs baseline: 1.2462x; 1.0288x over previous
"""Trainium2 Bass kernel for AllPassMORRCirculantLinear.

Math (reference, per batch row b):
  xb = x.reshape(bs, q, k); xb = xb*xb
  phi[b,p,q,t] = sum_s xb[b,q,s] * |w|[p,q,(t-s) mod k]   (circular conv, k=8)
  t(phi) = (a^2 + r^2 - 2 a r cos phi) / (1 + (ar)^2 - 2 a r cos phi)
  out[b, p*k+t] = sum_q scale[q] * t(phi[b,p,q,t])

With rho = a*r, A = (1-rho)^2, K = (1-a^2)(1-r^2), s'_q = -K*scale_q and
sum_q scale_q == 0:
  out = sum_q s'_q * u_q,  u_q = 1/(A + 4 rho sin^2(phi_q/2))

Pipeline (data-parallel over batch, 128 rows/core), per q:
  PE    : h = phi/(2pi) in fp32 PSUM via ONE 24-row stacked fp16 matmul
          per 512-chunk (rows = [xh, xl, xh] vs [wh, wh, wl]; weights
          pre-scaled by 1/2pi).  sin^2(pi*h) has period 1 in h, so range
          reduction is an EXACT integer subtract -- no Cody-Waite.
  DVE   : one fused 8-stage custom op SINSQ_FRAC_ANT:
            g = h - round(h)  (magic-number round, 3 stages)
            v = ((C0*g^2 + C1)*g)^2  (weighted-minimax squared-cubic
                ~ sin^2(pi g); its free additive constant rides the ACT
                reciprocal bias).  fp16 out.
  ACT   : u'_q = Reciprocal(SCL_q * v + BIA_q) = s'_q * u_q with
          SCL_q = 4 rho/s'_q, BIA_q = (A + 4 rho c)/s'_q baked as
          immediates (compile is keyed on the scale values); fp16 out.
          Square (x^2 staging) + Reciprocal live in ONE table set
          (reciprocal_and_small): zero ACT table switches.
  PE    : accumulation as identity-stationary matmuls into a PSUM
          region held across the iteration: acc[b,:] += I @ u'_q.
          fp16 moving operand streams 1 col/cycle; PSUM accumulates in
          fp32 (exact).  Readout = one ACT copy PSUM->SBUF per iter.
GPSIMD idle; DVE carries only the custom op; engines balance ~150us.
"""

import sys

for _p in ("/opt/trn_rl_repo",):
    if _p not in sys.path:
        sys.path.insert(0, _p)

import hashlib
import numpy as np
from contextlib import ExitStack

MRR_A = 0.8682
MRR_R = 0.8602
RHO = MRR_A * MRR_R
ACONST = (1.0 - RHO) ** 2
KCONST = (1.0 - MRR_A * MRR_A) * (1.0 - MRR_R * MRR_R)

BS, IN_CH, OUT_CH, KB = 1024, 1024, 1024, 8
Q = IN_CH // KB    # 128
P = OUT_CH // KB   # 128
NCORES = 8
BSC = BS // NCORES  # 128 batch rows per core

MAGIC = 12582912.0  # 1.5 * 2**23: y + MAGIC - MAGIC == round(y) in fp32 RNE
# weighted-minimax squared-cubic: sin^2(pi g) ~ (C1*g + C0*g^3)^2 + CB,
# weight 4 rho/d^2 (error measured in du/u); peak |du/u| = 7.9e-3
SINSQ_C0 = -4.64495414
SINSQ_C1 = 3.13652214
SINSQ_CB = 1.08338380e-05

# Accumulation split.  All-PE acc (identity matmuls) makes PE's rate ~=
# DVE's rate and the PE->DVE->ACT->PE loop then loses ~80us to coupling
# stalls (PE wait-queue is 4 deep; DVE has only 3 PSUM h-slots of
# buffer).  Giving the odd quads to the otherwise-idle GPSIMD keeps PE
# well under DVE's rate so the pipeline never starves: even quads of 4
# q's -> PE PSUM matmul acc (batched one group behind), odd quads ->
# ACT writes a [128, 4096] fp16 quad which GPSIMD adds into a 4-phase
# fp16 accumulator.
ACC_GROUP = 16
# which quads (mod 8, global index) go to GPSIMD; the rest go to PE.
# Alternating spreads the GP adds evenly -- clustered measures worse.
import os
GP_MASK8 = {
    "8":  (1, 5),
    "12": (1, 3, 5),
    "16": (1, 3, 5, 7),
}[os.environ.get("GP_QUADS", "12")]

_CACHE = {}


def _sinsq_ref(in0, in1, s0, s1, imm2):
    f = np.float32
    t = (in0.astype(f) + f(imm2)).astype(f)
    k = (t - f(imm2)).astype(f)
    g = (in0.astype(f) - k).astype(f)
    y = (g * g).astype(f)
    p = ((f(s0) * y).astype(f) + f(s1)).astype(f)
    e = (p * g).astype(f)
    return (e * e).astype(f)


def _register_sinsq():
    """Custom DVE op: v = ((C0*g^2 + C1)*g)^2, g = x - round(x). 8 stages."""
    from concourse import dve_ops
    from concourse.dve_spec import Spec, Src0, C0, C1, C2, sq, lower
    from concourse.dve_uop import DveOpSpec

    name = "SINSQ_FRAC_ANT"
    if name in dve_ops._SUB_OPCODE_FOR_NAME:
        return next(op for op in dve_ops.OPS if op.name == name)
    g = Src0 - ((Src0 + C2) - C2)
    e = (C0 * sq(g) + C1) * g
    spec = Spec(body=sq(e), reference=_sinsq_ref)
    row = max(dve_ops._SUB_OPCODE_FOR_NAME.values()) + 1
    assert row < 0x20
    dve_ops._SUB_OPCODE_FOR_NAME[name] = row
    shas = {}
    for ver in ("v3", "v4"):
        c = DveOpSpec(name=name, opcode=row, uops=lower(spec, ver=ver))
        shas[ver] = c.sha(ver)
    op = dve_ops.DveOp(name, spec, subdim=False, uops_sha=shas)
    dve_ops.OPS.append(op)
    dve_ops.CUSTOM_DVE_SPECS[name] = spec
    return op


def _raw_recip(nc, out, in_, bias, scale):
    """ACT Reciprocal (bypasses the bass low-precision guard; measured
    ~1.2e-5 max rel on TRN2 HW over the +-[0.3, 2000] range used here)."""
    from concourse import mybir

    eng = nc.scalar
    inputs = [eng.lower_ap(in_)]
    for arg in (bias, scale, 0.0):
        inputs.append(mybir.ImmediateValue(dtype=mybir.dt.float32, value=float(arg)))
    return eng.add_instruction(
        mybir.InstActivation(
            name=nc.get_next_instruction_name(),
            func=mybir.ActivationFunctionType.Reciprocal,
            ins=inputs,
            outs=[eng.lower_ap(out)],
        )
    )


def _build_nc(sprime, niter=1):
    from concourse import bacc, mybir
    import concourse.tile as tile
    from concourse import masks

    sinsq_op = _register_sinsq()

    nc = bacc.Bacc("TRN2", debug=False)
    f32 = mybir.dt.float32
    f16 = mybir.dt.float16
    AF = mybir.ActivationFunctionType

    x_d = nc.dram_tensor("x", [BSC, IN_CH], f32, kind="ExternalInput")
    # stacked circulant moving operand, pre-scaled by 1/(2 pi):
    # rows 0..7 wh (vs xh), 8..15 wh (vs xl), 16..23 wl (vs xh)
    wst_d = nc.dram_tensor("wst", [3 * KB, Q, OUT_CH], f16, kind="ExternalInput")
    out_d = nc.dram_tensor("out", [BSC, OUT_CH], f32, kind="ExternalOutput")

    # per-rail immediates (rails: s'_{q+64} = -s'_q exactly)
    assert np.array_equal(sprime[64:], -sprime[:64])
    scl = (4.0 * RHO / sprime[:64]).astype(np.float64)
    bia = ((ACONST + 4.0 * RHO * SINSQ_CB) / sprime[:64]).astype(np.float64)

    with tile.TileContext(nc) as tc:
        with ExitStack() as ctx:
            singles = ctx.enter_context(tc.tile_pool(name="singles", bufs=1))
            # h singles [128, 1024] = 2 banks * bufs=3 + acc 2 banks = 8
            psum = ctx.enter_context(tc.tile_pool(name="psum", bufs=3, space="PSUM"))
            apsum = ctx.enter_context(tc.tile_pool(name="apsum", bufs=1, space="PSUM"))
            wqp = ctx.enter_context(tc.tile_pool(name="wqp", bufs=2))
            vpool = ctx.enter_context(tc.tile_pool(name="vpool", bufs=6))
            upool = ctx.enter_context(tc.tile_pool(name="upool", bufs=12))
            gqpool = ctx.enter_context(tc.tile_pool(name="gqpool", bufs=4))

            identf = singles.tile([128, 128], f32)
            masks.make_identity(nc, identf[:])
            ident = singles.tile([128, 128], f16)
            nc.vector.tensor_copy(ident[:], identf[:])
            identn = singles.tile([128, 128], f16)
            nc.vector.tensor_scalar_mul(identn[:], identf[:], -1.0)

            x_sb = singles.tile([128, IN_CH], f32)
            nc.sync.dma_start(x_sb[:], x_d.ap())
            # input intensity modulation: x <- x^2 (in place).  Square is
            # in reciprocal_and_small: same table set as Reciprocal.
            nc.scalar.activation(x_sb[:], x_sb[:], AF.Square)

            # staged squared-transposed x in fp16 hi/lo (stationary):
            # rows 0..7 = xh, rows 8..15 = xl, rows 16..23 = xh (dup)
            xsts = []
            xlp = ctx.enter_context(tc.tile_pool(name="xlp", bufs=2))
            for g in range(8):
                xst = singles.tile([3 * KB, 16, 128], f16, tag=f"xst{g}")
                for half in range(2):
                    xtp = psum.tile([128, OUT_CH], f32, tag="h")
                    for j8 in range(8):
                        j = half * 8 + j8
                        nc.tensor.transpose(
                            xtp[0:8, j8 * 128:(j8 + 1) * 128],
                            x_sb[:, (g * 16 + j) * 8:(g * 16 + j) * 8 + 8],
                            identf[:])
                    xsl = xst[0:8, half * 8:(half + 1) * 8, :]
                    nc.scalar.copy(xsl, xtp[0:8, :].rearrange("s (j b) -> s j b", j=8))
                    xl_tmp = xlp.tile([8, 8 * 128], f16)
                    nc.vector.tensor_sub(xl_tmp[:], xtp[0:8, :],
                                         xsl.rearrange("s j b -> s (j b)"))
                    nc.scalar.dma_start(
                        xst[8:16, half * 8:(half + 1) * 8, :]
                        .rearrange("s j b -> s (j b)"), xl_tmp[:])
                    nc.scalar.dma_start(
                        xst[16:24, half * 8:(half + 1) * 8, :]
                        .rearrange("s j b -> s (j b)"),
                        xsl.rearrange("s j b -> s (j b)"))
                xsts.append(xst)

            out_sb = singles.tile([128, OUT_CH], f32)
            # GPSIMD-side 4-phase fp16 accumulator (odd quads)
            acc_g = singles.tile([128, 4 * OUT_CH], f16)

            wq_tiles = {}

            def wq_fetch(c):  # c = 8-q chunk index, 16 per iteration
                wq = wqp.tile([3 * KB, 8, OUT_CH], f16, tag="wq")
                nc.sync.dma_start(wq[:], wst_d.ap()[:, c * 8:(c + 1) * 8, :])
                wq_tiles[c % 2] = wq

            def h_sinsq(pos, v_dst):
                """PE h-matmul -> DVE sinsq into v_dst for one position.
                Positions are rail-paired: pos 2j <-> q=j, pos 2j+1 <->
                q=j+64 (wst is host-permuted to match)."""
                q = pos // 2 if pos % 2 == 0 else 64 + pos // 2
                gx, remx = divmod(q, 16)
                c, crem = divmod(pos, 8)
                if crem == 0 and c + 1 < 16:
                    wq_fetch(c + 1)  # prefetch next chunk (bufs=2)
                wq = wq_tiles[c % 2]
                hps = psum.tile([128, OUT_CH], f32, tag="h")
                for hc in range(2):
                    nc.tensor.matmul(
                        hps[:, hc * 512:(hc + 1) * 512],
                        xsts[gx][:, remx, :],
                        wq[:, crem, hc * 512:(hc + 1) * 512],
                        start=True, stop=True,
                    )
                nc.vector._custom_dve(
                    sinsq_op, out=v_dst, in0=hps[:],
                    s0=SINSQ_C0, s1=SINSQ_C1, imm2=MAGIC)

            def pair_front(p0, up_dst):
                """Two positions (one rail pair) -> one double-wide recip:
                up_dst[:, 0:1024] = u'_q, up_dst[:, 1024:2048] = -u'_{q+64}
                (s'_{q+64} = -s'_q exactly, and 1/(-x) = -1/x)."""
                vt = vpool.tile([128, 2 * OUT_CH], f16, tag="v")
                h_sinsq(p0, vt[:, 0:OUT_CH])
                h_sinsq(p0 + 1, vt[:, OUT_CH:2 * OUT_CH])
                rail = p0 // 2
                _raw_recip(nc, up_dst, vt[:], bias=bia[rail], scale=scl[rail])

            def run_iter():
                acc = apsum.tile([128, OUT_CH], f32, tag="acc")
                wq_fetch(0)
                ups = {}       # PE-destined u' pair tiles by pair index
                n_pe_total = 4 * (32 - 4 * len(GP_MASK8))  # positions on PE
                n_acc = [0]
                gp_first = True

                def acc_flush(before_pair):
                    """PE acc batch: all + halves (ident), then all -
                    halves (ident_neg) -- 2 stationary switches."""
                    pairs = [pr for pr in sorted(ups) if pr < before_pair]
                    for sgn in range(2):
                        stat = ident if sgn == 0 else identn
                        for pr in pairs:
                            up = ups[pr]
                            for hc in range(2):
                                # start/stop must hit the first/last matmul
                                # of EACH psum bank (hc 0 and 1)
                                nc.tensor.matmul(
                                    acc[:, hc * 512:(hc + 1) * 512],
                                    stat[:],
                                    up[:, sgn * OUT_CH + hc * 512:
                                       sgn * OUT_CH + (hc + 1) * 512],
                                    start=(n_acc[0] < 2),
                                    stop=(n_acc[0] >= 2 * n_pe_total - 2),
                                    skip_group_check=True,
                                )
                                n_acc[0] += 1
                    for pr in pairs:
                        ups.pop(pr)

                for g in range(Q // ACC_GROUP):
                    gbase = g * ACC_GROUP
                    for quad in range(ACC_GROUP // 4):
                        q0 = gbase + quad * 4
                        if (g * 4 + quad) % 8 not in GP_MASK8:
                            # PE quad: two u' pair tiles, acc'd next group
                            for pr in (q0 // 2, q0 // 2 + 1):
                                up = upool.tile([128, 2 * OUT_CH], f16,
                                                tag="u")
                                pair_front(2 * pr, up[:])
                                ups[pr] = up
                        else:
                            # GPSIMD quad: 2 rail pairs into one wide tile;
                            # phases 0,2 hold +u', phases 1,3 hold -u'
                            gq = gqpool.tile([128, 4 * OUT_CH], f16, tag="gq")
                            for j in range(2):
                                pair_front(q0 + 2 * j,
                                           gq[:, 2 * j * OUT_CH:
                                              2 * (j + 1) * OUT_CH])
                            if gp_first:
                                nc.gpsimd.tensor_copy(acc_g[:], gq[:])
                                gp_first = False
                            else:
                                nc.gpsimd.tensor_add(acc_g[:], acc_g[:], gq[:])
                    if g > 0:
                        acc_flush(gbase // 2)
                acc_flush(Q)
                nc.scalar.copy(out_sb[:], acc[:])

            if niter == 1:
                run_iter()
            else:
                with tc.For_i(0, niter, 1):
                    run_iter()

            # merge the GPSIMD-side phases (outside the loop): phases 0,2
            # hold +u', phases 1,3 hold -u'
            ph0 = singles.tile([128, OUT_CH], f32)
            ph1 = singles.tile([128, OUT_CH], f32)
            nc.vector.tensor_add(ph0[:], acc_g[:, 0:OUT_CH],
                                 acc_g[:, 2 * OUT_CH:3 * OUT_CH])
            nc.vector.tensor_add(ph1[:], acc_g[:, OUT_CH:2 * OUT_CH],
                                 acc_g[:, 3 * OUT_CH:4 * OUT_CH])
            nc.vector.tensor_sub(ph0[:], ph0[:], ph1[:])
            nc.vector.tensor_add(out_sb[:], out_sb[:], ph0[:])
            nc.sync.dma_start(out_d.ap(), out_sb[:])

    nc.compile()
    return nc


def _host_prep(weight, morr_output_scale):
    w = np.abs(np.asarray(weight, dtype=np.float32))   # [P, Q, KB]
    s = np.asarray(morr_output_scale, dtype=np.float32)
    s = s - s.mean()
    half = s[..., :-1, :]                              # [1,1,Q//2,1]
    scale = np.concatenate([half, -half], axis=2)[0, 0, :, 0].astype(np.float32)
    sprime = (-KCONST * scale).astype(np.float32)      # folded -K

    # circulant moving-operand layout: wc[s, q, p*KB+t] = w[p, q, (t-s) % KB],
    # pre-scaled by 1/(2 pi) so the matmul yields h = phi/(2 pi)
    wc = np.empty((KB, Q, P * KB), np.float32)
    for sh in range(KB):
        rolled = np.roll(w, sh, axis=2)
        wc[sh] = rolled.transpose(1, 0, 2).reshape(Q, P * KB)
    wc /= np.float32(2.0 * np.pi)

    wh = wc.astype(np.float16)
    wl = (wc - wh.astype(np.float32)).astype(np.float16)
    wst = np.empty((3 * KB, Q, P * KB), np.float16)
    wst[0:KB] = wh
    wst[KB:2 * KB] = wh
    wst[2 * KB:3 * KB] = wl
    # rail-paired position order: [0, 64, 1, 65, ...]
    perm = np.empty(Q, np.int64)
    perm[0::2] = np.arange(Q // 2)
    perm[1::2] = np.arange(Q // 2) + Q // 2
    wst = np.ascontiguousarray(wst[:, perm, :])
    return wst, sprime


def kernel(x, weight, morr_output_scale, _trace=False):
    from concourse import bass_utils

    wst, sprime = _host_prep(weight, morr_output_scale)
    key = hashlib.sha1(sprime.tobytes()).hexdigest()
    if _CACHE.get("key") != key:
        _CACHE["nc"] = _build_nc(sprime)
        _CACHE["key"] = key
    nc = _CACHE["nc"]

    x = np.ascontiguousarray(np.asarray(x, dtype=np.float32))
    in_maps = []
    for c in range(NCORES):
        in_maps.append({
            "x": np.ascontiguousarray(x[c * BSC:(c + 1) * BSC]),
            "wst": wst,
        })
    res = bass_utils.run_bass_kernel_spmd(
        nc, in_maps, core_ids=list(range(NCORES)), trace=_trace)
    out = np.concatenate([res.results[c]["out"] for c in range(NCORES)], axis=0)
    if _trace:
        _CACHE["last_results"] = res
    return out
